# revision 43
# baseline (speedup 1.0000x reference)
import os
import sys
import numpy as np
from contextlib import ExitStack

sys.path.insert(0, "/opt/trn_rl_repo")

from concourse import mybir  # noqa: E402
from concourse import bass, tile  # noqa: E402
from concourse import bass_utils  # noqa: E402

# Static problem geometry (hardcoded per contract)
B, P, M, K = 32, 8192, 64, 32
C_FEAT, C_IN, E = 13, 16, 384
EPS = 1e-5
NCORES = 8
CPC = B // NCORES          # clouds per core = 4
NEG = -3.0e38              # finite -inf substitute (sim requires finite)

F32 = mybir.dt.float32
F16 = mybir.dt.float16
I16 = mybir.dt.int16
U16 = mybir.dt.uint16
AF = mybir.ActivationFunctionType


def _build_nc(sim_gelu=False, stage="full"):
    # sim_gelu: CoreSim lacks Gelu; substitute x*sigmoid(1.702x) for the
    # sim-only correctness gate (test.py compares vs a matching numpy model)
    # stage: "A"|"B"|"C"|"full" — HW bisection cutoffs (debug output "dbg")
    nc = bass.Bass()
    xq_h = nc.declare_dram_parameter("xq", [2, 12, P], F32, isOutput=False)
    d2l_h = nc.declare_dram_parameter("d2l", [2, 12, 128], F32, isOutput=False)
    tabt_h = nc.declare_dram_parameter("tabt", [128, P], F16, isOutput=False)
    b1s_h = nc.declare_dram_parameter("b1s", [128, 8, 64], F16, isOutput=False)
    eye_h = nc.declare_dram_parameter("eye", [128, 512], F16, isOutput=False)
    w1t_h = nc.declare_dram_parameter("w1t", [128, 64], F16, isOutput=False)
    w2t_h = nc.declare_dram_parameter("w2t", [64, 128], F16, isOutput=False)
    w3t_h = nc.declare_dram_parameter("w3t", [128, 3, 128], F16, isOutput=False)
    b2c_h = nc.declare_dram_parameter("b2c", [128, 1], F32, isOutput=False)
    b3c_h = nc.declare_dram_parameter("b3c", [128, 3], F32, isOutput=False)
    tok_h = nc.declare_dram_parameter("tok", [CPC, 3, 128, 64], F32, isOutput=True)
    idxd_h = nc.dram_tensor("idxd", [CPC, 64, 32], I16)  # bounce scratch
    dbg_h = None
    if stage != "full":
        dbg_h = nc.declare_dram_parameter("dbg", [128, CPC * 128], I16,
                                          isOutput=True)

    with tile.TileContext(nc) as tc, ExitStack() as ctx:
        # long-lived single tiles
        def T(shape, dtype, name):
            t, f = tc.tile(shape, dtype, name=name)
            ctx.callback(f)
            return t

        w1t = T([128, 64], F16, "w1t_sb")
        w2t = T([64, 128], F16, "w2t_sb")
        w3t = T([128, 3, 128], F16, "w3t_sb")
        eye = T([128, 512], F16, "eye_sb")
        b1s = T([128, 8, 64], F16, "b1s_sb")
        b2c = T([128, 1], F32, "b2c_sb")
        b3c = T([128, 3], F32, "b3c_sb")
        d2l = T([12, 2, 128], F32, "d2l_sb")
        xqs = T([12, 2, P], F32, "xq_sb")
        negd2 = T([128, P], F32, "negd2_sb")
        vals8 = T([128, 8], F32, "vals8_sb")
        idx_sb = T([128, 64], U16, "idx_sb")
        tabt = T([128, P], F16, "tabt_sb")
        idxw = T([128, 128], U16, "idxw_sb")
        gat = T([128, 2048], F16, "gat_sb")
        gmv = T([16, 2048], F16, "gmv_sb")

        pd2 = ctx.enter_context(tc.tile_pool(name="pd2", bufs=2, space="PSUM"))
        pp1 = ctx.enter_context(tc.tile_pool(name="pp1", bufs=2, space="PSUM"))
        pp2 = ctx.enter_context(tc.tile_pool(name="pp2", bufs=2, space="PSUM"))
        pp3 = ctx.enter_context(tc.tile_pool(name="pp3", bufs=2, space="PSUM"))
        ph1 = ctx.enter_context(tc.tile_pool(name="ph1", bufs=2))
        ph2 = ctx.enter_context(tc.tile_pool(name="ph2", bufs=2))
        ptk = ctx.enter_context(tc.tile_pool(name="ptk", bufs=2))
        ptmp = ctx.enter_context(tc.tile_pool(name="ptmp", bufs=2))

        def gelu_act(out_ap, z_ap, part, bias_ap=None):
            if not sim_gelu:
                if bias_ap is None:
                    nc.scalar.activation(out_ap, z_ap, AF.Gelu)
                else:
                    nc.scalar.activation(out_ap, z_ap, AF.Gelu, bias=bias_ap)
                return
            zb = ptmp.tile([part, 512], F32)
            if bias_ap is None:
                nc.vector.tensor_copy(zb[:], z_ap)
            else:
                nc.vector.tensor_scalar_add(zb[:], z_ap, bias_ap)
            sg = ptmp.tile([part, 512], F32)
            nc.scalar.activation(sg[:], zb[:], AF.Sigmoid, scale=1.702)
            nc.vector.tensor_mul(out_ap, zb[:], sg[:])

        # const loads
        nc.sync.dma_start(out=w1t[:], in_=w1t_h[:])
        nc.sync.dma_start(out=w2t[:], in_=w2t_h[:])
        nc.sync.dma_start(out=w3t[:], in_=w3t_h[:])
        nc.sync.dma_start(out=eye[:], in_=eye_h[:])
        nc.sync.dma_start(out=b1s[:], in_=b1s_h[:])
        nc.sync.dma_start(out=b2c[:], in_=b2c_h[:])
        nc.sync.dma_start(out=b3c[:], in_=b3c_h[:])
        nc.sync.dma_start(out=d2l[:], in_=d2l_h.rearrange("r t m -> t r m"))
        nc.sync.dma_start(out=tabt[:], in_=tabt_h[:])
        nc.vector.memset(idxw[:], 0)

        # ---- phase A: -d2' matmul + top-32 per pair ----
        for pr in range(2):
            nc.sync.dma_start(out=xqs[:, pr], in_=xq_h[pr])
            for t in range(16):
                ps = pd2.tile([128, 512], F32)
                nc.tensor.matmul(ps[:], d2l[:, pr], xqs[:, pr, t * 512:(t + 1) * 512],
                                 start=True, stop=True)
                dst = negd2[:, t * 512:(t + 1) * 512]
                if t % 2 == 0:
                    nc.scalar.activation(dst, ps[:], AF.Copy)
                else:
                    nc.vector.tensor_copy(dst, ps[:])
            for r in range(4):
                nc.vector.max(vals8[:], negd2[:])
                nc.vector.max_index(idx_sb[:, pr * 32 + r * 8: pr * 32 + r * 8 + 8],
                                    vals8[:], negd2[:])
                if r < 3:
                    nc.vector.match_replace(negd2[:], vals8[:], negd2[:], NEG)
            # bounce idx (clouds 2pr, 2pr+1) to DRAM
            nc.sync.dma_start(
                out=idxd_h[2 * pr: 2 * pr + 2].rearrange("c m k -> (c m) k"),
                in_=idx_sb[:, pr * 32: pr * 32 + 32].bitcast(I16),
            )

        if stage == "A":
            nc.sync.dma_start(out=dbg_h[:, 0:64], in_=idx_sb[:].bitcast(I16))
            return nc

        # ---- phase B: wrapped-16 per-group index readback ----
        # group g=2c (partitions 32c..32c+16) gathers cloud c; i = m*32+k
        # idxw[32c + k%16, 2m + k//16] = idxd[c, m, k]
        for c in range(CPC):
            nc.sync.dma_start(
                out=idxw[32 * c: 32 * c + 16, :].bitcast(I16).rearrange(
                    "kl (m kh) -> kl m kh", m=64, kh=2),
                in_=idxd_h[c].rearrange("m (kh kl) -> kl m kh", kh=2, kl=16),
            )

        if stage == "B":
            nc.sync.dma_start(out=dbg_h[:, 0:128], in_=idxw[:].bitcast(I16))
            return nc

        # ---- phase C: SBUF free-dim gather (per-16-partition-group idxs) ----
        for s in range(0, 2048, 512):
            nc.gpsimd.indirect_copy(gat[:, s:s + 512], tabt[:],
                                    idxw[:, s // 16:(s + 512) // 16], True)
        # PE rhs base partition must be 0/32/64 — move cloud 3 (base 96) to 0
        nc.sync.dma_start(out=gmv[:], in_=gat[96:112, :])

        if stage == "C":
            nc.sync.dma_start(out=dbg_h[:], in_=gat[:, 0:512].bitcast(I16))
            return nc

        # ---- phase D: MLP + maxpool per cloud ----
        for c in range(CPC):
            tok = ptk.tile([128, 3, 64], F32)
            for b in range(4):
                if c < 3:
                    g0, bcol = 32 * c, b
                    rhs1 = gat[g0:g0 + 16, b * 512:(b + 1) * 512]
                else:
                    g0, bcol = 0, 4 + b
                    rhs1 = gmv[:, b * 512:(b + 1) * 512]
                ps1 = pp1.tile([64, 512], F32)
                nc.tensor.matmul(ps1[:], w1t[g0:g0 + 16], rhs1,
                                 start=True, stop=False)
                nc.tensor.matmul(ps1[:], b1s[g0:g0 + 16, bcol], eye[g0:g0 + 16],
                                 start=False, stop=True)
                h1 = ph1.tile([64, 512], F16)
                gelu_act(h1[:], ps1[:], 64)
                ps2 = pp2.tile([128, 512], F32)
                nc.tensor.matmul(ps2[:], w2t[:], h1[:], start=True, stop=True)
                h2 = ph2.tile([128, 512], F16)
                gelu_act(h2[:], ps2[:], 128, bias_ap=b2c[:, 0:1])
                for t in range(3):
                    ps3 = pp3.tile([128, 512], F32)
                    nc.tensor.matmul(ps3[:], w3t[:, t], h2[:], start=True, stop=True)
                    nc.vector.tensor_reduce(
                        tok[:, t, b * 16:(b + 1) * 16],
                        ps3[:].rearrange("p (q k) -> p q k", q=16, k=32),
                        axis=mybir.AxisListType.X, op=mybir.AluOpType.max)
            for t in range(3):
                nc.vector.tensor_scalar_add(tok[:, t], tok[:, t], b3c[:, t:t + 1])
            nc.sync.dma_start(out=tok_h[c].rearrange("t p m -> p t m"), in_=tok[:])

    return nc


def _fold(W, b, g, be, m, v):
    s = g / np.sqrt(v + EPS)
    return (W * s[:, None]).astype(np.float32), ((b - m) * s + be).astype(np.float32)


def _prep(inputs):
    xyz = np.asarray(inputs["xyz"], np.float32)
    feat = np.asarray(inputs["features"], np.float32)
    pts = xyz.reshape(B, P, 3)
    fts = feat.reshape(B, P, C_FEAT)
    centers = pts[:, ::P // M].copy()  # (B,64,3)

    Wf1, bf1 = _fold(*(np.asarray(inputs[k], np.float32) for k in
                       ("W1", "b1", "g1", "be1", "m1", "v1")))
    Wf2, bf2 = _fold(*(np.asarray(inputs[k], np.float32) for k in
                       ("W2", "b2", "g2", "be2", "m2", "v2")))
    Wf3, bf3 = _fold(*(np.asarray(inputs[k], np.float32) for k in
                       ("W3", "b3", "g3", "be3", "m3", "v3")))

    eye16 = np.zeros((16, 512), np.float16)
    eye16[np.arange(512) // 32, np.arange(512)] = 1.0
    eye_rep = np.zeros((128, 512), np.float16)
    w1t_rep = np.zeros((128, 64), np.float16)
    for c in range(CPC):
        eye_rep[32 * c: 32 * c + 16] = eye16
        w1t_rep[32 * c: 32 * c + 16] = Wf1.T.astype(np.float16)
    shared = dict(
        eye=eye_rep,
        w1t=w1t_rep,
        w2t=np.ascontiguousarray(Wf2.T).astype(np.float16),
        w3t=np.ascontiguousarray(Wf3.T.reshape(128, 3, 128)).astype(np.float16),
        b2c=bf2.reshape(128, 1).astype(np.float32),
        b3c=np.ascontiguousarray(bf3.reshape(3, 128).T).astype(np.float32),
    )

    in_maps = []
    for ci in range(NCORES):
        p4 = pts[4 * ci: 4 * ci + 4]      # (4,8192,3)
        f4 = fts[4 * ci: 4 * ci + 4]
        c4 = centers[4 * ci: 4 * ci + 4]  # (4,64,3)
        xq = np.zeros((2, 12, P), np.float32)
        d2l = np.zeros((2, 12, 128), np.float32)
        for pr in range(2):
            for h in range(2):
                c = 2 * pr + h
                xyzT = p4[c].T
                xq[pr, 6 * h: 6 * h + 3] = xyzT
                xq[pr, 6 * h + 3: 6 * h + 6] = xyzT * xyzT
                d2l[pr, 6 * h: 6 * h + 3, 64 * h: 64 * h + 64] = 2.0 * c4[c].T
                d2l[pr, 6 * h + 3: 6 * h + 6, 64 * h: 64 * h + 64] = -1.0
        tabt = np.zeros((128, P), np.float16)
        for c in range(CPC):
            tabt[32 * c: 32 * c + 3] = p4[c].T.astype(np.float16)
            tabt[32 * c + 3: 32 * c + 16] = f4[c].T.astype(np.float16)
        shift = np.einsum('cmd,od->cmo', c4, Wf1[:, :3])       # (4,64,64)
        resh = (bf1[None, None] - shift).reshape(CPC, 4, 16, 64)
        b1s = np.zeros((128, 8, 64), np.float16)
        for c in range(CPC):
            b1s[32 * c: 32 * c + 16, 0:4] = resh[c].transpose(1, 0, 2)
        b1s[0:16, 4:8] = resh[3].transpose(1, 0, 2)
        m = dict(xq=xq, d2l=d2l, tabt=tabt, b1s=b1s)
        m.update(shared)
        in_maps.append(m)
    return in_maps, centers


def _legalize_waits(nc):
    # This walrus codegen accepts at most ONE sync wait per instruction
    # (Matmult LW struct and even NoOp CTRL struct reject 2+). Hoist the
    # excess waits onto preceding same-engine NoOps, one wait each;
    # engine program order preserves the sync semantics.
    for f in nc.m.functions:
        for bb in f.blocks:
            lst = list(bb.instructions)
            out = []
            changed = False
            for inst in lst:
                si = inst.sync_info
                if si is not None and len(si.on_wait) > 1:
                    waits = list(si.on_wait)
                    mm = isinstance(inst, mybir.InstMatmult)
                    keep = [] if mm else waits[-1:]
                    hoist = waits if mm else waits[:-1]
                    for w in hoist:
                        nop = mybir.InstNoOp(
                            name=nc.get_next_instruction_name(),
                            engine=inst.engine,
                            sync_info=mybir.SyncInfo(
                                on_wait=[w], on_update=[]),
                            bass_nofuse=True,
                        )
                        nc.register_instruction(nop)
                        out.append(nop)
                    si.on_wait = keep
                    changed = True
                elif (isinstance(inst, mybir.InstMatmult)
                        and si is not None and si.on_wait):
                    nop = mybir.InstNoOp(
                        name=nc.get_next_instruction_name(),
                        engine=inst.engine,
                        sync_info=mybir.SyncInfo(
                            on_wait=list(si.on_wait), on_update=[]),
                        bass_nofuse=True,
                    )
                    nc.register_instruction(nop)
                    si.on_wait = []
                    out.append(nop)
                    changed = True
                out.append(inst)
            if changed:
                bb.instructions = out


_NC_CACHE = None
LAST_EXEC_TIME_NS = None


def kernel(**inputs):
    global _NC_CACHE, LAST_EXEC_TIME_NS
    in_maps, centers = _prep(inputs)
    if _NC_CACHE is None:
        _NC_CACHE = _build_nc()
        _legalize_waits(_NC_CACHE)
        # dma_gather/load_library are extended InstISA pseudo-instructions;
        # raw Bass needs this pass to populate .instr bytes before neuronxcc
        # ("ISA wrong length" otherwise).
        mybir.codegen_inst_isa_subclasses(_NC_CACHE)
    trace = bool(int(os.environ.get("KERNEL_TRACE", "0")))
    res = bass_utils.run_bass_kernel_spmd(
        _NC_CACHE, in_maps, list(range(NCORES)), trace=trace)
    LAST_EXEC_TIME_NS = res.exec_time_ns
    if LAST_EXEC_TIME_NS is None:
        # no ntff trace under axon: report best wall-clock of warm reruns
        # (includes host->device staging, so an upper bound on exec time)
        import time
        best = None
        for _ in range(3):
            t0 = time.perf_counter()
            bass_utils.run_bass_kernel_spmd(
                _NC_CACHE, in_maps, list(range(NCORES)), trace=False)
            dt = time.perf_counter() - t0
            best = dt if best is None or dt < best else best
        LAST_EXEC_TIME_NS = int(best * 1e9)
    tokens = np.zeros((B, M, E), np.float32)
    for ci in range(NCORES):
        t4 = res.results[ci]["tok"]  # (4,3,128,64)
        tokens[4 * ci: 4 * ci + 4] = np.ascontiguousarray(
            t4.transpose(0, 3, 1, 2)).reshape(CPC, 64, E)
    return tokens, centers
